# revision 4
# baseline (speedup 1.0000x reference)
"""Trainium2 Bass kernel for nn_NodeModel (gnn_message_passing).

Reference computation:
    agg = segment_sum(edge_attr, edge_index[0], N)   # [N, 64]
    h   = relu(concat([x, agg], 1) @ W1 + b1)        # [N, 256]
    out = h @ W2 + b2                                # [N, 64]
(u and batch are unused by the reference.)

Strategy (8 cores, graph-parallel, slot-aligned scatter):
  * Host sorts nodes by degree (desc) and deals them into 128-node
    windows: global stripe of 1024 sorted nodes -> 128 nodes per core.
    Window k's depth D_k = max degree in its stripe; because nodes in a
    stripe have nearly equal degree, padding is ~1%.
  * Edge e with row endpoint r is stored at [partition p(r), column
    off_k + depth_rank(e)] of its core's edge buffer, as fp8-e3m4
    (4-bit mantissa: rel-err ~1.2e-2 end-to-end; e4m3 measured 2.6e-2,
    over the 2e-2 gate).  Aggregation is then a plain sum of aligned
    [128 nodes, 64 feat] slices -- no one-hot matmuls, no per-chunk
    PE weight loads:
      - PE windows: accumulate slices into PSUM with matmuls against a
        CONSTANT fp8 identity (stationary stays loaded / FWL).
      - DVE windows: vector tensor_reduce over a [128, 64, D] strided
        view (innermost = depth).
    The split keeps both engines busy; DMA of edge bytes (12.8 MB/core
    at 358 GB/s ~= 36 us) is the roofline.
  * agg is node-major; a PE transpose per window moves it into the
    feature-major catT buffer (rows 0:64 agg, 64:128 x) for the MLP.
  * MLP per 512-node supertile in bf16: hT = relu(W1.T @ catT + b1),
    outT = W2.T @ hT + b2; relu halves split ACT/DVE; out bias+copy on
    DVE; outputs DMA'd as bf16.
"""

import os
import sys

for _p in ("/opt/trn_rl_repo", "/root/.axon_site/_ro/trn_rl_repo"):
    if os.path.isdir(_p) and _p not in sys.path:
        sys.path.insert(0, _p)

import numpy as np
import ml_dtypes
from contextlib import ExitStack

import concourse.bass as bass
import concourse.tile as tile
from concourse import bacc, mybir
from concourse.bass_utils import run_bass_kernel_spmd

F32 = mybir.dt.float32
F32R = mybir.dt.float32r
BF16 = mybir.dt.bfloat16
F8 = mybir.dt.float8e3
U8 = mybir.dt.uint8
E3M4 = ml_dtypes.float8_e3m4

NCORES = 8
D = 64            # feature dim
H = 256           # hidden dim
O = 64            # output dim
W = 128           # nodes per window (= partitions)
GW = 4            # windows per supertile group
ST = GW * W       # 512-node MLP supertile
STRIPE = NCORES * W
DVE_FRAC = float(os.environ.get("K_DVE_FRAC", "0.55"))


class Cfg:
    """Static per-NEFF structure, identical across cores."""

    def __init__(self, depths):
        self.D = tuple(int(d) for d in depths)       # depth per window
        self.K = len(self.D)                         # windows per core
        self.NPC = W * self.K
        off = np.concatenate([[0], np.cumsum(self.D)]).astype(np.int64)
        self.off = tuple(int(o) for o in off)
        self.S = int(off[-1])                        # total slices per core
        self.NGRP = self.K // GW
        # per-group DVE/PE assignment: within each group give the deepest
        # windows to DVE until ~DVE_FRAC of the group's slices are covered.
        dve = []
        for g in range(self.NGRP):
            ks = list(range(g * GW, (g + 1) * GW))
            ks.sort(key=lambda k: -self.D[k])
            tot = sum(self.D[k] for k in ks)
            acc = 0
            for k in ks:
                if acc < DVE_FRAC * tot:
                    dve.append(k)
                    acc += self.D[k]
        self.dve = frozenset(dve)
        self.SGmax = max(
            sum(self.D[g * GW + j] for j in range(GW))
            for g in range(self.NGRP))

    def key(self):
        return (self.D, sorted(self.dve))


# ----------------------------------------------------------------- host pack

def _structure(n_nodes, deg_sorted):
    K = max(1, int(np.ceil(n_nodes / STRIPE)))
    K = ((K + GW - 1) // GW) * GW
    degs = np.zeros(K * STRIPE, np.int64)
    degs[:n_nodes] = deg_sorted
    depths = np.maximum(degs.reshape(K, STRIPE).max(1), 1)
    return Cfg(depths)


def _pack(x, edge_index, edge_attr, W1, b1, W2, b2):
    n_nodes = x.shape[0]
    n_edges = edge_attr.shape[0]
    row = np.asarray(edge_index[0], np.int64)
    deg = np.bincount(row, minlength=n_nodes)
    order = np.argsort(-deg, kind="stable")
    cfg = _structure(n_nodes, deg[order])
    off = np.asarray(cfg.off, np.int64)

    node_rank = np.empty(n_nodes, np.int64)
    node_rank[order] = np.arange(n_nodes)
    k_of = node_rank // STRIPE                 # window index
    r_in = node_rank % STRIPE
    c_of = r_in // W                           # core
    p_of = r_in % W                            # partition

    # depth rank of each edge within its row node
    eidx = np.argsort(row, kind="stable")
    srow = row[eidx]
    d_sorted = np.arange(n_edges) - np.searchsorted(srow, srow)
    drank = np.empty(n_edges, np.int64)
    drank[eidx] = d_sorted

    ce, ke, pe = c_of[row], k_of[row], p_of[row]
    assert (drank < np.asarray(cfg.D)[ke]).all()
    cole = off[ke] + drank

    q = np.asarray(edge_attr, np.float32).astype(E3M4).view(np.uint8)
    EB = np.zeros((NCORES, W, cfg.S, D), np.uint8)
    EB[ce, pe, cole] = q

    # node features, feature-major per core
    slots = np.zeros((NCORES, cfg.NPC, D), np.float32)
    slots[c_of, k_of * W + p_of] = np.asarray(x, np.float32)
    xT = np.ascontiguousarray(
        slots.transpose(0, 2, 1)).astype(ml_dtypes.bfloat16)

    perm = np.full(NCORES * cfg.NPC, -1, np.int64)
    perm[c_of * cfg.NPC + k_of * W + p_of] = np.arange(n_nodes)
    mask = perm >= 0

    # catT rows 0:64 hold agg, 64:128 hold x -> swap W1's row halves
    W1f = np.asarray(W1, np.float32)
    W1p = np.concatenate([W1f[D:2 * D], W1f[0:D]], axis=0).astype(
        ml_dtypes.bfloat16)                               # [128, 256]
    W2p = np.ascontiguousarray(
        np.asarray(W2, np.float32).reshape(2, 128, O).transpose(1, 0, 2)
        .reshape(128, 2 * O)).astype(ml_dtypes.bfloat16)  # [128, 128]
    b1T = np.ascontiguousarray(
        np.asarray(b1, np.float32).reshape(2, 128).T)     # [128, 2]
    b2c = np.asarray(b2, np.float32).reshape(O, 1)        # [64, 1]

    I8 = np.eye(W, dtype=np.float32).astype(E3M4).view(np.uint8)
    Ib = np.eye(W, dtype=np.float32).astype(ml_dtypes.bfloat16)
    Ifr = np.eye(W, dtype=np.float32)

    in_maps = []
    for c in range(NCORES):
        in_maps.append({
            "edges": EB[c], "xT": xT[c],
            "W1": W1p, "W2p": W2p, "b1T": b1T, "b2": b2c,
            "I8": I8, "Ib": Ib, "Ifr": Ifr,
        })
    return in_maps, cfg, perm, mask


# -------------------------------------------------------------- device build

def build_nc(cfg, reps=1, skip=frozenset()):
    nc = bacc.Bacc("TRN2", target_bir_lowering=False, debug=False)
    ap_edges = nc.dram_tensor("edges", [W, cfg.S, D], U8,
                              kind="ExternalInput").ap()
    ap_xT = nc.dram_tensor("xT", [D, cfg.NPC], BF16,
                           kind="ExternalInput").ap()
    ap_W1 = nc.dram_tensor("W1", [2 * D, H], BF16, kind="ExternalInput").ap()
    ap_W2p = nc.dram_tensor("W2p", [H // 2, 2 * O], BF16,
                            kind="ExternalInput").ap()
    ap_b1T = nc.dram_tensor("b1T", [H // 2, 2], F32,
                            kind="ExternalInput").ap()
    ap_b2 = nc.dram_tensor("b2", [O, 1], F32, kind="ExternalInput").ap()
    ap_I8 = nc.dram_tensor("I8", [W, W], U8, kind="ExternalInput").ap()
    ap_Ib = nc.dram_tensor("Ib", [W, W], BF16, kind="ExternalInput").ap()
    ap_Ifr = nc.dram_tensor("Ifr", [W, W], F32R, kind="ExternalInput").ap()
    ap_out = nc.dram_tensor("outT", [O, cfg.NPC], BF16,
                            kind="ExternalOutput").ap()

    AF = mybir.ActivationFunctionType
    with tile.TileContext(nc) as tc, ExitStack() as ctx:
        consts = ctx.enter_context(tc.tile_pool(name="consts", bufs=1))
        epool = ctx.enter_context(tc.tile_pool(name="edges", bufs=3))
        abpool = ctx.enter_context(tc.tile_pool(name="agg_bf", bufs=4))
        afpool = ctx.enter_context(tc.tile_pool(name="agg_fr", bufs=4))
        hpool = ctx.enter_context(tc.tile_pool(name="hid", bufs=3))
        ypool = ctx.enter_context(tc.tile_pool(name="yout", bufs=2))
        ps_a = ctx.enter_context(tc.tile_pool(name="ps_agg", bufs=2,
                                              space="PSUM"))
        ps_t = ctx.enter_context(tc.tile_pool(name="ps_tr", bufs=2,
                                              space="PSUM"))
        ps_h = ctx.enter_context(tc.tile_pool(name="ps_h", bufs=2,
                                              space="PSUM"))
        ps_o = ctx.enter_context(tc.tile_pool(name="ps_o", bufs=2,
                                              space="PSUM"))

        catT = consts.tile([2 * D, cfg.NPC], BF16)
        nc.sync.dma_start(catT[D:2 * D, :], ap_xT)
        W1t = consts.tile([2 * D, H], BF16)
        nc.sync.dma_start(W1t[:], ap_W1)
        W2t = consts.tile([H // 2, 2 * O], BF16)
        nc.sync.dma_start(W2t[:], ap_W2p)
        b1T = consts.tile([H // 2, 2], F32)
        nc.sync.dma_start(b1T[:], ap_b1T)
        b2t = consts.tile([O, 1], F32)
        nc.sync.dma_start(b2t[:], ap_b2)
        I8t = consts.tile([W, W], U8)
        nc.sync.dma_start(I8t[:], ap_I8)
        Ibt = consts.tile([W, W], BF16)
        nc.sync.dma_start(Ibt[:], ap_Ib)
        Ifrt = consts.tile([W, W], F32R)
        nc.sync.dma_start(Ifrt[:], ap_Ifr)
        I8v = I8t[:].bitcast(F8)

        def mlp(st):
            cat_sl = catT[:, st * ST:(st + 1) * ST]
            hs = []
            for half in range(2):
                w1h = W1t[:, half * 128:(half + 1) * 128]
                h_ps = ps_h.tile([128, ST], F32, tag="h_ps")
                nc.tensor.matmul(h_ps[:], w1h, cat_sl, start=True, stop=True)
                h_sb = hpool.tile([128, ST], BF16, tag="h_sb")
                if half == 0:
                    nc.scalar.activation(h_sb[:], h_ps[:], AF.Relu,
                                         bias=b1T[:, 0:1])
                else:
                    nc.vector.tensor_scalar(
                        h_sb[:], h_ps[:], b1T[:, 1:2], 0.0,
                        op0=mybir.AluOpType.add, op1=mybir.AluOpType.max)
                hs.append(h_sb)
            o_ps = ps_o.tile([O, ST], F32)
            nc.tensor.matmul(o_ps[:], W2t[:, 0:O], hs[0][:],
                             start=True, stop=False)
            nc.tensor.matmul(o_ps[:], W2t[:, O:2 * O], hs[1][:],
                             start=False, stop=True)
            o_sb = ypool.tile([O, ST], BF16)
            nc.vector.tensor_scalar_add(o_sb[:], o_ps[:], b2t[:, 0:1])
            nc.sync.dma_start(ap_out[:, st * ST:(st + 1) * ST], o_sb[:])

        for rep in range(reps):
            for g in range(cfg.NGRP):
                goff = cfg.off[g * GW]
                SG = cfg.off[(g + 1) * GW] - goff
                et = epool.tile([W, cfg.SGmax, D], U8, tag="et")
                if "edma" not in skip:
                    nc.sync.dma_start(et[:, 0:SG, :],
                                      ap_edges[:, goff:goff + SG, :])
                elif rep == 0 and g == 0:
                    nc.vector.memset(et[:], 0)
                a_ps = ps_a.tile([W, GW, D], F32, tag="a_ps")
                for j in range(GW):
                    k = g * GW + j
                    woff = cfg.off[k] - goff
                    Dk = cfg.D[k]
                    if "red" in skip:
                        continue
                    if k in cfg.dve:
                        agg = afpool.tile([W, D], F32R, tag="agg_fr")
                        src = (et[:, woff:woff + Dk, :]
                               .transpose([0, 2, 1]).bitcast(F8))
                        with nc.allow_low_precision(
                                reason="f32r out, fp32-width accumulate"):
                            nc.vector.tensor_reduce(
                                agg[:], src, axis=mybir.AxisListType.X,
                                op=mybir.AluOpType.add)
                        tr = ps_t.tile([D, W], F32R, tag="tr")
                        nc.tensor.transpose(tr[:], agg[:], Ifrt[:])
                    else:
                        for d in range(Dk):
                            nc.tensor.matmul(
                                a_ps[:, j, :], I8v,
                                et[:, woff + d, :].bitcast(F8),
                                start=(d == 0), stop=(d == Dk - 1))
                        agg = abpool.tile([W, D], F32R, tag="agg_bf")
                        nc.scalar.activation(agg[:], a_ps[:, j, :], AF.Copy)
                        tr = ps_t.tile([D, W], F32R, tag="tr")
                        nc.tensor.transpose(tr[:], agg[:], Ifrt[:])
                    nc.scalar.activation(catT[0:D, k * W:(k + 1) * W],
                                         tr[:], AF.Copy)
                if "mlp" not in skip:
                    mlp(g)
    nc.compile()
    return nc


# ------------------------------------------------------------------- driver

_CACHE = {}


def prepare(inputs, reps=1, skip=frozenset()):
    in_maps, cfg, perm, mask = _pack(
        np.asarray(inputs["x"]), np.asarray(inputs["edge_index"]),
        np.asarray(inputs["edge_attr"]),
        inputs["W1"], inputs["b1"], inputs["W2"], inputs["b2"])
    key = (cfg.key()[0], tuple(cfg.key()[1]), reps, tuple(sorted(skip)))
    if key not in _CACHE:
        _CACHE[key] = build_nc(cfg, reps=reps, skip=skip)
    return _CACHE[key], in_maps, cfg, perm, mask


def unpack_out(results, cfg, perm, mask, n_nodes):
    slots = np.concatenate(
        [np.asarray(r["outT"]).astype(np.float32).T for r in results], axis=0)
    y = np.zeros((n_nodes, O), np.float32)
    y[perm[mask]] = slots[mask]
    return y


def kernel(**inputs):
    nc, in_maps, cfg, perm, mask = prepare(inputs)
    res = run_bass_kernel_spmd(nc, in_maps, list(range(NCORES)))
    return unpack_out(res.results, cfg, perm, mask,
                      np.asarray(inputs["x"]).shape[0])


# revision 6
# speedup vs baseline: 444.5253x; 444.5253x over previous
"""Trainium2 Bass kernel for nn_NodeModel (gnn_message_passing).

Reference computation:
    agg = segment_sum(edge_attr, edge_index[0], N)   # [N, 64]
    h   = relu(concat([x, agg], 1) @ W1 + b1)        # [N, 256]
    out = h @ W2 + b2                                # [N, 64]
(u and batch are unused by the reference.)

Strategy (8 cores, graph-parallel, slot-aligned scatter):
  * Host sorts nodes by degree (desc) and deals them into 128-node
    windows: global stripe of 1024 sorted nodes -> 128 nodes per core.
    Window k's depth D_k = max degree in its stripe; because nodes in a
    stripe have nearly equal degree, padding is ~1%.
  * Edge e with row endpoint r is stored at [partition p(r), column
    off_k + depth_rank(e)] of its core's edge buffer, as fp8-e3m4
    (4-bit mantissa: rel-err ~1.2e-2 end-to-end; e4m3 measured 2.6e-2,
    over the 2e-2 gate).  Aggregation is then a plain sum of aligned
    [128 nodes, 64 feat] slices -- no one-hot matmuls, no per-chunk
    PE weight loads:
      - PE windows: accumulate slices into PSUM with matmuls against a
        CONSTANT fp8 identity (stationary stays loaded / FWL).
      - DVE windows: vector tensor_reduce over a [128, 64, D] strided
        view (innermost = depth).
    The split keeps both engines busy; DMA of edge bytes (12.8 MB/core
    at 358 GB/s ~= 36 us) is the roofline.
  * agg is node-major; a PE transpose per window moves it into the
    feature-major catT buffer (rows 0:64 agg, 64:128 x) for the MLP.
  * MLP per 512-node supertile in bf16: hT = relu(W1.T @ catT + b1),
    outT = W2.T @ hT + b2; relu halves split ACT/DVE; out bias+copy on
    DVE; outputs DMA'd as bf16.
"""

import os
import sys

for _p in ("/opt/trn_rl_repo", "/root/.axon_site/_ro/trn_rl_repo"):
    if os.path.isdir(_p) and _p not in sys.path:
        sys.path.insert(0, _p)

import numpy as np
import ml_dtypes
from contextlib import ExitStack

import concourse.bass as bass
import concourse.tile as tile
from concourse import bacc, mybir
from concourse.bass_utils import run_bass_kernel_spmd

F32 = mybir.dt.float32
F32R = mybir.dt.float32r
BF16 = mybir.dt.bfloat16
F8 = mybir.dt.float8e3
U8 = mybir.dt.uint8
E3M4 = ml_dtypes.float8_e3m4

NCORES = 8
D = 64            # feature dim
H = 256           # hidden dim
O = 64            # output dim
W = 128           # nodes per window (= partitions)
GW = 4            # windows per supertile group
ST = GW * W       # 512-node MLP supertile
STRIPE = NCORES * W
DVE_FRAC = float(os.environ.get("K_DVE_FRAC", "0.55"))


class Cfg:
    """Static per-NEFF structure, identical across cores."""

    def __init__(self, depths):
        self.D = tuple(int(d) for d in depths)       # depth per window
        self.K = len(self.D)                         # windows per core
        self.NPC = W * self.K
        off = np.concatenate([[0], np.cumsum(self.D)]).astype(np.int64)
        self.off = tuple(int(o) for o in off)
        self.S = int(off[-1])                        # total slices per core
        self.NGRP = self.K // GW
        # per-group DVE/PE assignment: within each group give the deepest
        # windows to DVE until ~DVE_FRAC of the group's slices are covered.
        dve = []
        for g in range(self.NGRP):
            ks = list(range(g * GW, (g + 1) * GW))
            ks.sort(key=lambda k: -self.D[k])
            tot = sum(self.D[k] for k in ks)
            acc = 0
            for k in ks:
                if acc < DVE_FRAC * tot:
                    dve.append(k)
                    acc += self.D[k]
        self.dve = frozenset(dve)
        self.SGmax = max(
            sum(self.D[g * GW + j] for j in range(GW))
            for g in range(self.NGRP))

    def key(self):
        return (self.D, sorted(self.dve))


# ----------------------------------------------------------------- host pack

def _structure(n_nodes, deg_sorted):
    K = max(1, int(np.ceil(n_nodes / STRIPE)))
    K = ((K + GW - 1) // GW) * GW
    degs = np.zeros(K * STRIPE, np.int64)
    degs[:n_nodes] = deg_sorted
    depths = np.maximum(degs.reshape(K, STRIPE).max(1), 1)
    return Cfg(depths)


def _pack(x, edge_index, edge_attr, W1, b1, W2, b2):
    n_nodes = x.shape[0]
    n_edges = edge_attr.shape[0]
    row = np.asarray(edge_index[0], np.int64)
    deg = np.bincount(row, minlength=n_nodes)
    order = np.argsort(-deg, kind="stable")
    cfg = _structure(n_nodes, deg[order])
    off = np.asarray(cfg.off, np.int64)

    node_rank = np.empty(n_nodes, np.int64)
    node_rank[order] = np.arange(n_nodes)
    k_of = node_rank // STRIPE                 # window index
    r_in = node_rank % STRIPE
    c_of = r_in // W                           # core
    p_of = r_in % W                            # partition

    # depth rank of each edge within its row node
    eidx = np.argsort(row, kind="stable")
    srow = row[eidx]
    d_sorted = np.arange(n_edges) - np.searchsorted(srow, srow)
    drank = np.empty(n_edges, np.int64)
    drank[eidx] = d_sorted

    ce, ke, pe = c_of[row], k_of[row], p_of[row]
    assert (drank < np.asarray(cfg.D)[ke]).all()
    cole = off[ke] + drank

    q = np.asarray(edge_attr, np.float32).astype(E3M4).view(np.uint8)
    EB = np.zeros((NCORES, W, cfg.S, D), np.uint8)
    EB[ce, pe, cole] = q

    # node features, feature-major per core
    slots = np.zeros((NCORES, cfg.NPC, D), np.float32)
    slots[c_of, k_of * W + p_of] = np.asarray(x, np.float32)
    xT = np.ascontiguousarray(
        slots.transpose(0, 2, 1)).astype(ml_dtypes.bfloat16)

    perm = np.full(NCORES * cfg.NPC, -1, np.int64)
    perm[c_of * cfg.NPC + k_of * W + p_of] = np.arange(n_nodes)
    mask = perm >= 0

    # catT rows 0:64 hold agg, 64:128 hold x -> swap W1's row halves
    W1f = np.asarray(W1, np.float32)
    W1p = np.concatenate([W1f[D:2 * D], W1f[0:D]], axis=0).astype(
        ml_dtypes.bfloat16)                               # [128, 256]
    W2p = np.ascontiguousarray(
        np.asarray(W2, np.float32).reshape(2, 128, O).transpose(1, 0, 2)
        .reshape(128, 2 * O)).astype(ml_dtypes.bfloat16)  # [128, 128]
    b1T = np.ascontiguousarray(
        np.asarray(b1, np.float32).reshape(2, 128).T)     # [128, 2]
    b2c = np.asarray(b2, np.float32).reshape(O, 1)        # [64, 1]

    I8 = np.eye(W, dtype=np.float32).astype(E3M4).view(np.uint8)
    Ib = np.eye(W, dtype=np.float32).astype(ml_dtypes.bfloat16)
    Ifr = np.eye(W, dtype=np.float32)

    in_maps = []
    for c in range(NCORES):
        in_maps.append({
            "edges": EB[c], "xT": xT[c],
            "W1": W1p, "W2p": W2p, "b1T": b1T, "b2": b2c,
            "I8": I8, "Ib": Ib, "Ifr": Ifr,
        })
    return in_maps, cfg, perm, mask


# -------------------------------------------------------------- device build

def build_nc(cfg, reps=1, skip=frozenset()):
    nc = bacc.Bacc("TRN2", target_bir_lowering=False, debug=False)
    ap_edges = nc.dram_tensor("edges", [W, cfg.S, D], U8,
                              kind="ExternalInput").ap()
    ap_xT = nc.dram_tensor("xT", [D, cfg.NPC], BF16,
                           kind="ExternalInput").ap()
    ap_W1 = nc.dram_tensor("W1", [2 * D, H], BF16, kind="ExternalInput").ap()
    ap_W2p = nc.dram_tensor("W2p", [H // 2, 2 * O], BF16,
                            kind="ExternalInput").ap()
    ap_b1T = nc.dram_tensor("b1T", [H // 2, 2], F32,
                            kind="ExternalInput").ap()
    ap_b2 = nc.dram_tensor("b2", [O, 1], F32, kind="ExternalInput").ap()
    ap_I8 = nc.dram_tensor("I8", [W, W], U8, kind="ExternalInput").ap()
    ap_Ib = nc.dram_tensor("Ib", [W, W], BF16, kind="ExternalInput").ap()
    ap_Ifr = nc.dram_tensor("Ifr", [W, W], F32R, kind="ExternalInput").ap()
    ap_out = nc.dram_tensor("outT", [O, cfg.NPC], BF16,
                            kind="ExternalOutput").ap()

    AF = mybir.ActivationFunctionType
    with tile.TileContext(nc) as tc, ExitStack() as ctx:
        consts = ctx.enter_context(tc.tile_pool(name="consts", bufs=1))
        epool = ctx.enter_context(tc.tile_pool(name="edges", bufs=3))
        abpool = ctx.enter_context(tc.tile_pool(name="agg_bf", bufs=4))
        afpool = ctx.enter_context(tc.tile_pool(name="agg_fr", bufs=4))
        hpool = ctx.enter_context(tc.tile_pool(name="hid", bufs=3))
        ypool = ctx.enter_context(tc.tile_pool(name="yout", bufs=2))
        ps_a = ctx.enter_context(tc.tile_pool(name="ps_agg", bufs=2,
                                              space="PSUM"))
        ps_t = ctx.enter_context(tc.tile_pool(name="ps_tr", bufs=2,
                                              space="PSUM"))
        ps_h = ctx.enter_context(tc.tile_pool(name="ps_h", bufs=2,
                                              space="PSUM"))
        ps_o = ctx.enter_context(tc.tile_pool(name="ps_o", bufs=2,
                                              space="PSUM"))

        catT = consts.tile([2 * D, cfg.NPC], BF16)
        nc.sync.dma_start(catT[D:2 * D, :], ap_xT)
        W1t = consts.tile([2 * D, H], BF16)
        nc.sync.dma_start(W1t[:], ap_W1)
        W2t = consts.tile([H // 2, 2 * O], BF16)
        nc.sync.dma_start(W2t[:], ap_W2p)
        b1T = consts.tile([H // 2, 2], F32)
        nc.sync.dma_start(b1T[:], ap_b1T)
        b2t = consts.tile([O, 1], F32)
        nc.sync.dma_start(b2t[:], ap_b2)
        I8t = consts.tile([W, W], U8)
        nc.sync.dma_start(I8t[:], ap_I8)
        Ibt = consts.tile([W, W], BF16)
        nc.sync.dma_start(Ibt[:], ap_Ib)
        Ifrt = consts.tile([W, W], F32R)
        nc.sync.dma_start(Ifrt[:], ap_Ifr)
        I8v = I8t[:].bitcast(F8)

        def mlp(st):
            cat_sl = catT[:, st * ST:(st + 1) * ST]
            hs = []
            for half in range(2):
                w1h = W1t[:, half * 128:(half + 1) * 128]
                h_ps = ps_h.tile([128, ST], F32, tag="h_ps")
                nc.tensor.matmul(h_ps[:], w1h, cat_sl, start=True, stop=True)
                h_sb = hpool.tile([128, ST], BF16, tag="h_sb")
                if half == 0:
                    nc.scalar.activation(h_sb[:], h_ps[:], AF.Relu,
                                         bias=b1T[:, 0:1])
                else:
                    nc.vector.tensor_scalar(
                        h_sb[:], h_ps[:], b1T[:, 1:2], 0.0,
                        op0=mybir.AluOpType.add, op1=mybir.AluOpType.max)
                hs.append(h_sb)
            o_ps = ps_o.tile([O, ST], F32)
            nc.tensor.matmul(o_ps[:], W2t[:, 0:O], hs[0][:],
                             start=True, stop=False)
            nc.tensor.matmul(o_ps[:], W2t[:, O:2 * O], hs[1][:],
                             start=False, stop=True)
            o_sb = ypool.tile([O, ST], BF16)
            nc.vector.tensor_scalar_add(o_sb[:], o_ps[:], b2t[:, 0:1])
            nc.sync.dma_start(ap_out[:, st * ST:(st + 1) * ST], o_sb[:])

        for rep in range(reps):
            for g in range(cfg.NGRP):
                goff = cfg.off[g * GW]
                SG = cfg.off[(g + 1) * GW] - goff
                et = epool.tile([W, cfg.SGmax, D], U8, tag="et")
                if "edma" not in skip:
                    nc.sync.dma_start(et[:, 0:SG, :],
                                      ap_edges[:, goff:goff + SG, :])
                elif rep == 0 and g == 0:
                    nc.vector.memset(et[:], 0)
                for j in range(GW):
                    k = g * GW + j
                    woff = cfg.off[k] - goff
                    Dk = cfg.D[k]
                    if "red" in skip:
                        continue
                    if k in cfg.dve:
                        agg = afpool.tile([W, D], F32R, tag="agg_fr")
                        src = (et[:, woff:woff + Dk, :]
                               .transpose([0, 2, 1]).bitcast(F8))
                        with nc.allow_low_precision(
                                reason="f32r out, fp32-width accumulate"):
                            nc.vector.tensor_reduce(
                                agg[:], src, axis=mybir.AxisListType.X,
                                op=mybir.AluOpType.add)
                        tr = ps_t.tile([D, W], F32R, tag="tr")
                        nc.tensor.transpose(tr[:], agg[:], Ifrt[:])
                    else:
                        a_ps = ps_a.tile([W, D], F32, tag="a_ps")
                        for d in range(Dk):
                            nc.tensor.matmul(
                                a_ps[:], I8v,
                                et[:, woff + d, :].bitcast(F8),
                                start=(d == 0), stop=(d == Dk - 1))
                        agg = abpool.tile([W, D], F32R, tag="agg_bf")
                        nc.scalar.activation(agg[:], a_ps[:], AF.Copy)
                        tr = ps_t.tile([D, W], F32R, tag="tr")
                        nc.tensor.transpose(tr[:], agg[:], Ifrt[:])
                    nc.scalar.activation(catT[0:D, k * W:(k + 1) * W],
                                         tr[:], AF.Copy)
                if "mlp" not in skip:
                    mlp(g)
    nc.compile()
    return nc


# ------------------------------------------------------------------- driver

_CACHE = {}


def prepare(inputs, reps=1, skip=frozenset()):
    in_maps, cfg, perm, mask = _pack(
        np.asarray(inputs["x"]), np.asarray(inputs["edge_index"]),
        np.asarray(inputs["edge_attr"]),
        inputs["W1"], inputs["b1"], inputs["W2"], inputs["b2"])
    key = (cfg.key()[0], tuple(cfg.key()[1]), reps, tuple(sorted(skip)))
    if key not in _CACHE:
        _CACHE[key] = build_nc(cfg, reps=reps, skip=skip)
    return _CACHE[key], in_maps, cfg, perm, mask


def unpack_out(results, cfg, perm, mask, n_nodes):
    slots = np.concatenate(
        [np.asarray(r["outT"]).astype(np.float32).T for r in results], axis=0)
    y = np.zeros((n_nodes, O), np.float32)
    y[perm[mask]] = slots[mask]
    return y


def kernel(**inputs):
    nc, in_maps, cfg, perm, mask = prepare(inputs)
    res = run_bass_kernel_spmd(nc, in_maps, list(range(NCORES)))
    return unpack_out(res.results, cfg, perm, mask,
                      np.asarray(inputs["x"]).shape[0])


# revision 8
# speedup vs baseline: 831.4537x; 1.8704x over previous
"""Trainium2 Bass kernel for nn_NodeModel (gnn_message_passing).

Reference computation:
    agg = segment_sum(edge_attr, edge_index[0], N)   # [N, 64]
    h   = relu(concat([x, agg], 1) @ W1 + b1)        # [N, 256]
    out = h @ W2 + b2                                # [N, 64]
(u and batch are unused by the reference.)

Strategy (8 cores, graph-parallel, slot-aligned scatter):
  * Host sorts nodes by degree (desc) and deals them into 128-node
    windows: global stripe of 1024 sorted nodes -> 128 nodes per core.
    Window k's depth D_k = max degree in its stripe; because nodes in a
    stripe have nearly equal degree, padding is ~1%.
  * Edge e with row endpoint r is stored at [partition p(r), column
    off_k + depth_rank(e)] of its core's edge buffer, as fp8-e3m4
    (4-bit mantissa: rel-err ~1.2e-2 end-to-end; e4m3 measured 2.6e-2,
    over the 2e-2 gate).  Aggregation is then a plain sum of aligned
    [128 nodes, 64 feat] slices -- no one-hot matmuls, no per-chunk
    PE weight loads:
      - PE windows: accumulate slices into PSUM with matmuls against a
        CONSTANT fp8 identity (stationary stays loaded / FWL).
      - DVE windows: vector tensor_reduce over a [128, 64, D] strided
        view (innermost = depth).
    The split keeps both engines busy; DMA of edge bytes (12.8 MB/core
    at 358 GB/s ~= 36 us) is the roofline.
  * agg is node-major; a PE transpose per window moves it into the
    feature-major catT buffer (rows 0:64 agg, 64:128 x) for the MLP.
  * MLP per 512-node supertile in bf16: hT = relu(W1.T @ catT + b1),
    outT = W2.T @ hT + b2; relu halves split ACT/DVE; out bias+copy on
    DVE; outputs DMA'd as bf16.
"""

import os
import sys

for _p in ("/opt/trn_rl_repo", "/root/.axon_site/_ro/trn_rl_repo"):
    if os.path.isdir(_p) and _p not in sys.path:
        sys.path.insert(0, _p)

import numpy as np
import ml_dtypes
from contextlib import ExitStack

import concourse.bass as bass
import concourse.tile as tile
from concourse import bacc, mybir
from concourse.bass_utils import run_bass_kernel_spmd

F32 = mybir.dt.float32
F32R = mybir.dt.float32r
BF16 = mybir.dt.bfloat16
F8 = mybir.dt.float8e3
U8 = mybir.dt.uint8
E3M4 = ml_dtypes.float8_e3m4

NCORES = 8
D = 64            # feature dim
H = 256           # hidden dim
O = 64            # output dim
W = 128           # nodes per window (= partitions)
GW = 4            # windows per supertile group
ST = GW * W       # 512-node MLP supertile
STRIPE = NCORES * W
DVE_FRAC = float(os.environ.get("K_DVE_FRAC", "0.55"))


class Cfg:
    """Static per-NEFF structure, identical across cores."""

    def __init__(self, depths):
        self.D = tuple(int(d) for d in depths)       # depth per window
        self.K = len(self.D)                         # windows per core
        self.NPC = W * self.K
        off = np.concatenate([[0], np.cumsum(self.D)]).astype(np.int64)
        self.off = tuple(int(o) for o in off)
        self.S = int(off[-1])                        # total slices per core
        self.NGRP = self.K // GW
        # per-group DVE/PE assignment: within each group give the deepest
        # windows to DVE until ~DVE_FRAC of the group's slices are covered.
        dve = []
        for g in range(self.NGRP):
            ks = list(range(g * GW, (g + 1) * GW))
            ks.sort(key=lambda k: -self.D[k])
            tot = sum(self.D[k] for k in ks)
            acc = 0
            for k in ks:
                if acc < DVE_FRAC * tot:
                    dve.append(k)
                    acc += self.D[k]
        self.dve = frozenset(dve)
        self.SGmax = max(
            sum(self.D[g * GW + j] for j in range(GW))
            for g in range(self.NGRP))

    def key(self):
        return (self.D, sorted(self.dve))


# ----------------------------------------------------------------- host pack

def _structure(n_nodes, deg_sorted):
    K = max(1, int(np.ceil(n_nodes / STRIPE)))
    K = ((K + GW - 1) // GW) * GW
    degs = np.zeros(K * STRIPE, np.int64)
    degs[:n_nodes] = deg_sorted
    depths = np.maximum(degs.reshape(K, STRIPE).max(1), 1)
    return Cfg(depths)


def _pack(x, edge_index, edge_attr, W1, b1, W2, b2):
    n_nodes = x.shape[0]
    n_edges = edge_attr.shape[0]
    row = np.asarray(edge_index[0], np.int64)
    deg = np.bincount(row, minlength=n_nodes)
    order = np.argsort(-deg, kind="stable")
    cfg = _structure(n_nodes, deg[order])
    off = np.asarray(cfg.off, np.int64)

    node_rank = np.empty(n_nodes, np.int64)
    node_rank[order] = np.arange(n_nodes)
    k_of = node_rank // STRIPE                 # window index
    r_in = node_rank % STRIPE
    c_of = r_in // W                           # core
    p_of = r_in % W                            # partition

    # depth rank of each edge within its row node
    eidx = np.argsort(row, kind="stable")
    srow = row[eidx]
    d_sorted = np.arange(n_edges) - np.searchsorted(srow, srow)
    drank = np.empty(n_edges, np.int64)
    drank[eidx] = d_sorted

    ce, ke, pe = c_of[row], k_of[row], p_of[row]
    assert (drank < np.asarray(cfg.D)[ke]).all()
    cole = off[ke] + drank

    q = np.asarray(edge_attr, np.float32).astype(E3M4).view(np.uint8)
    EB = np.zeros((NCORES, W, cfg.S, D), np.uint8)
    EB[ce, pe, cole] = q
    # DVE windows are stored feature-major (depth innermost/contiguous) so
    # the on-device tensor_reduce reads unit-stride.
    for k in sorted(cfg.dve):
        a, b = cfg.off[k], cfg.off[k + 1]
        blk = EB[:, :, a:b, :]                     # [8, 128, Dk, 64]
        EB[:, :, a:b, :] = np.ascontiguousarray(
            blk.transpose(0, 1, 3, 2)).reshape(blk.shape)

    # node features, feature-major per core
    slots = np.zeros((NCORES, cfg.NPC, D), np.float32)
    slots[c_of, k_of * W + p_of] = np.asarray(x, np.float32)
    xT = np.ascontiguousarray(
        slots.transpose(0, 2, 1)).astype(ml_dtypes.bfloat16)

    perm = np.full(NCORES * cfg.NPC, -1, np.int64)
    perm[c_of * cfg.NPC + k_of * W + p_of] = np.arange(n_nodes)
    mask = perm >= 0

    # catT rows 0:64 hold agg, 64:128 hold x -> swap W1's row halves
    W1f = np.asarray(W1, np.float32)
    W1p = np.concatenate([W1f[D:2 * D], W1f[0:D]], axis=0).astype(
        ml_dtypes.bfloat16)                               # [128, 256]
    W2p = np.ascontiguousarray(
        np.asarray(W2, np.float32).reshape(2, 128, O).transpose(1, 0, 2)
        .reshape(128, 2 * O)).astype(ml_dtypes.bfloat16)  # [128, 128]
    b1T = np.ascontiguousarray(
        np.asarray(b1, np.float32).reshape(2, 128).T)     # [128, 2]
    b2c = np.asarray(b2, np.float32).reshape(O, 1)        # [64, 1]

    I8 = np.eye(W, dtype=np.float32).astype(E3M4).view(np.uint8)
    Ib = np.eye(W, dtype=np.float32).astype(ml_dtypes.bfloat16)
    Ifr = np.eye(W, dtype=np.float32)

    in_maps = []
    for c in range(NCORES):
        in_maps.append({
            "edges": EB[c], "xT": xT[c],
            "W1": W1p, "W2p": W2p, "b1T": b1T, "b2": b2c,
            "I8": I8, "Ib": Ib, "Ifr": Ifr,
        })
    return in_maps, cfg, perm, mask


# -------------------------------------------------------------- device build

def build_nc(cfg, reps=1, skip=frozenset()):
    nc = bacc.Bacc("TRN2", target_bir_lowering=False, debug=False)
    ap_edges = nc.dram_tensor("edges", [W, cfg.S, D], U8,
                              kind="ExternalInput").ap()
    ap_xT = nc.dram_tensor("xT", [D, cfg.NPC], BF16,
                           kind="ExternalInput").ap()
    ap_W1 = nc.dram_tensor("W1", [2 * D, H], BF16, kind="ExternalInput").ap()
    ap_W2p = nc.dram_tensor("W2p", [H // 2, 2 * O], BF16,
                            kind="ExternalInput").ap()
    ap_b1T = nc.dram_tensor("b1T", [H // 2, 2], F32,
                            kind="ExternalInput").ap()
    ap_b2 = nc.dram_tensor("b2", [O, 1], F32, kind="ExternalInput").ap()
    ap_I8 = nc.dram_tensor("I8", [W, W], U8, kind="ExternalInput").ap()
    ap_Ib = nc.dram_tensor("Ib", [W, W], BF16, kind="ExternalInput").ap()
    ap_Ifr = nc.dram_tensor("Ifr", [W, W], F32R, kind="ExternalInput").ap()
    ap_out = nc.dram_tensor("outT", [O, cfg.NPC], BF16,
                            kind="ExternalOutput").ap()

    AF = mybir.ActivationFunctionType
    with tile.TileContext(nc) as tc, ExitStack() as ctx:
        consts = ctx.enter_context(tc.tile_pool(name="consts", bufs=1))
        epool = ctx.enter_context(tc.tile_pool(name="edges", bufs=3))
        abpool = ctx.enter_context(tc.tile_pool(name="agg_bf", bufs=4))
        afpool = ctx.enter_context(tc.tile_pool(name="agg_fr", bufs=4))
        hpool = ctx.enter_context(tc.tile_pool(name="hid", bufs=3))
        ypool = ctx.enter_context(tc.tile_pool(name="yout", bufs=2))
        ps_a = ctx.enter_context(tc.tile_pool(name="ps_agg", bufs=2,
                                              space="PSUM"))
        ps_t = ctx.enter_context(tc.tile_pool(name="ps_tr", bufs=2,
                                              space="PSUM"))
        ps_h = ctx.enter_context(tc.tile_pool(name="ps_h", bufs=2,
                                              space="PSUM"))
        ps_o = ctx.enter_context(tc.tile_pool(name="ps_o", bufs=2,
                                              space="PSUM"))

        catT = consts.tile([2 * D, cfg.NPC], BF16)
        nc.sync.dma_start(catT[D:2 * D, :], ap_xT)
        W1t = consts.tile([2 * D, H], BF16)
        nc.sync.dma_start(W1t[:], ap_W1)
        W2t = consts.tile([H // 2, 2 * O], BF16)
        nc.sync.dma_start(W2t[:], ap_W2p)
        b1T = consts.tile([H // 2, 2], F32)
        nc.sync.dma_start(b1T[:], ap_b1T)
        b2t = consts.tile([O, 1], F32)
        nc.sync.dma_start(b2t[:], ap_b2)
        I8t = consts.tile([W, W], U8)
        nc.sync.dma_start(I8t[:], ap_I8)
        Ibt = consts.tile([W, W], BF16)
        nc.sync.dma_start(Ibt[:], ap_Ib)
        Ifrt = consts.tile([W, W], F32R)
        nc.sync.dma_start(Ifrt[:], ap_Ifr)
        I8v = I8t[:].bitcast(F8)

        def mlp(st):
            cat_sl = catT[:, st * ST:(st + 1) * ST]
            hs = []
            for half in range(2):
                w1h = W1t[:, half * 128:(half + 1) * 128]
                h_ps = ps_h.tile([128, ST], F32, tag="h_ps")
                nc.tensor.matmul(h_ps[:], w1h, cat_sl, start=True, stop=True)
                h_sb = hpool.tile([128, ST], BF16, tag="h_sb")
                if half == 0:
                    nc.scalar.activation(h_sb[:], h_ps[:], AF.Relu,
                                         bias=b1T[:, 0:1])
                else:
                    nc.vector.tensor_scalar(
                        h_sb[:], h_ps[:], b1T[:, 1:2], 0.0,
                        op0=mybir.AluOpType.add, op1=mybir.AluOpType.max)
                hs.append(h_sb)
            o_ps = ps_o.tile([O, ST], F32)
            nc.tensor.matmul(o_ps[:], W2t[:, 0:O], hs[0][:],
                             start=True, stop=False)
            nc.tensor.matmul(o_ps[:], W2t[:, O:2 * O], hs[1][:],
                             start=False, stop=True)
            o_sb = ypool.tile([O, ST], BF16)
            nc.vector.tensor_scalar_add(o_sb[:], o_ps[:], b2t[:, 0:1])
            nc.sync.dma_start(ap_out[:, st * ST:(st + 1) * ST], o_sb[:])

        for rep in range(reps):
            for g in range(cfg.NGRP):
                goff = cfg.off[g * GW]
                SG = cfg.off[(g + 1) * GW] - goff
                et = epool.tile([W, cfg.SGmax, D], U8, tag="et")
                if "edma" not in skip:
                    nc.sync.dma_start(et[:, 0:SG, :],
                                      ap_edges[:, goff:goff + SG, :])
                elif rep == 0 and g == 0:
                    nc.vector.memset(et[:], 0)
                for j in range(GW):
                    k = g * GW + j
                    woff = cfg.off[k] - goff
                    Dk = cfg.D[k]
                    if "red" in skip:
                        continue
                    if k in cfg.dve:
                        agg = afpool.tile([W, D], F32R, tag="agg_fr")
                        src = (et[:, woff:woff + Dk, :]
                               .rearrange("p a b -> p (a b)")
                               .rearrange("p (f d) -> p f d", d=Dk)
                               .bitcast(F8))
                        with nc.allow_low_precision(
                                reason="f32r out, fp32-width accumulate"):
                            nc.vector.tensor_reduce(
                                agg[:], src, axis=mybir.AxisListType.X,
                                op=mybir.AluOpType.add)
                        tr = ps_t.tile([D, W], F32R, tag="tr")
                        nc.tensor.transpose(tr[:], agg[:], Ifrt[:])
                    else:
                        a_ps = ps_a.tile([W, D], F32, tag="a_ps")
                        for d in range(Dk):
                            nc.tensor.matmul(
                                a_ps[:], I8v,
                                et[:, woff + d, :].bitcast(F8),
                                start=(d == 0), stop=(d == Dk - 1))
                        agg = abpool.tile([W, D], F32R, tag="agg_bf")
                        nc.scalar.activation(agg[:], a_ps[:], AF.Copy)
                        tr = ps_t.tile([D, W], F32R, tag="tr")
                        nc.tensor.transpose(tr[:], agg[:], Ifrt[:])
                    nc.scalar.activation(catT[0:D, k * W:(k + 1) * W],
                                         tr[:], AF.Copy)
                if "mlp" not in skip:
                    mlp(g)
    nc.compile()
    return nc


# ------------------------------------------------------------------- driver

_CACHE = {}


def prepare(inputs, reps=1, skip=frozenset()):
    in_maps, cfg, perm, mask = _pack(
        np.asarray(inputs["x"]), np.asarray(inputs["edge_index"]),
        np.asarray(inputs["edge_attr"]),
        inputs["W1"], inputs["b1"], inputs["W2"], inputs["b2"])
    key = (cfg.key()[0], tuple(cfg.key()[1]), reps, tuple(sorted(skip)))
    if key not in _CACHE:
        _CACHE[key] = build_nc(cfg, reps=reps, skip=skip)
    return _CACHE[key], in_maps, cfg, perm, mask


def unpack_out(results, cfg, perm, mask, n_nodes):
    slots = np.concatenate(
        [np.asarray(r["outT"]).astype(np.float32).T for r in results], axis=0)
    y = np.zeros((n_nodes, O), np.float32)
    y[perm[mask]] = slots[mask]
    return y


def kernel(**inputs):
    nc, in_maps, cfg, perm, mask = prepare(inputs)
    res = run_bass_kernel_spmd(nc, in_maps, list(range(NCORES)))
    return unpack_out(res.results, cfg, perm, mask,
                      np.asarray(inputs["x"]).shape[0])
